# revision 16
# baseline (speedup 1.0000x reference)
"""GCL (GNN message-passing) Trainium2 Bass kernel on 8 NeuronCores.

Sharding: edges sorted by destination on host and sharded by destination-node
range (1250 nodes/core) -> each core owns the full segment-sum for its nodes,
no collectives. Node features and weights replicated.

Per core, edges are processed in 512-edge macros grouped in 1024-edge pairs
(one dma_gather call + one PSUM [128,1024] tile pair per phase):
  e1T[D,e] = A @ S_T + gathered-B^T                (PSUM, all-bf16 matmuls)
  where A = h@we1_top (bf16, SBUF resident per-window),
        B = h@we1_bot (bf16 HBM table, per-edge transpose-dma_gather on col
        lands directly in [D,e] layout, injected via one identity matmul),
        S_T[n,e] = one-hot(row_local[e]==n) via DVE is_equal (bf16 in/out).
  e1s = silu(e1T + be1) via Act bias arg (one [128,1024] silu per pair)
  e2[e,D] = silu(e1s @ we2 + be2)                  (bias via K=1 bf16 matmul)
  aggT[D,n] += e2^T-scatter via lhsT=e2s, rhs=S4   (PSUM accumulate per window)
Node MLP + residual per 128-node tile, bf16 weights, f32 residual.
1/NORM folded into wn1_hi on host; be1/bn1 folded into Act silu bias.
"""
import sys
sys.path.insert(0, '/opt/trn_rl_repo')
import numpy as np
import ml_dtypes

N_NODES = 10000
N_EDGES = 640000
D = 128
NORM = 100.0
NCORES = 8
NPC = N_NODES // NCORES          # 1250 destination nodes per core
NWIN = 10                        # 128-node windows per core
MACRO = 512
CALL = 1024                      # edges per dma_gather call (= 1 macro pair)
PAD_ROWLOCAL = 200.0

BF16 = ml_dtypes.bfloat16
_prog_cache = {}


def _wrap_idx16(idx):
    """[n] int -> [128, n/16] int16 wrapped (pos i -> partition i%16, col
    i//16) and replicated into all eight 16-partition groups."""
    n = idx.shape[0]
    block = idx.astype(np.int16).reshape(n // 16, 16).T
    return np.tile(block, (8, 1))


QPAT = [0, 1, 0, 2, 0, 1, 0, 3]   # queue shares ~ 1/cost: cost_q ~ (q+1)


def _build_program(mw_per_window, bufs_g=8, bufs_w=3,
                   no_gather=False, no_compute=False):
    import concourse.bacc as bacc
    import concourse.mybir as mybir
    from concourse import tile

    dt = mybir.dt
    AF = mybir.ActivationFunctionType
    ALU = mybir.AluOpType

    NM = sum(mw_per_window)          # total 512-edge macros (even)
    assert NM % 2 == 0
    NP = NM // 2                     # pairs == gather calls

    nc = bacc.Bacc("TRN2", target_bir_lowering=False, debug=False,
                   num_devices=NCORES, num_swdge_queues=4)

    f32, bf16, i16 = dt.float32, dt.bfloat16, dt.int16
    din = lambda n, s, d=f32: nc.dram_tensor(n, s, d, kind="ExternalInput")
    hT_bf = din("hT_bf", [128, 10240], bf16)
    hTs_bf = din("hTs_bf", [128, NWIN * 128], bf16)
    h_slice = din("h_slice", [NWIN, 128, 128])
    we1_top = din("we1_top", [128, 128], bf16)
    we1_bot = din("we1_bot", [128, 128], bf16)
    be1_col = din("be1_col", [128, 1])
    we2_d = din("we2", [128, 128], bf16)
    be2rep4 = din("be2rep4", [1, 512], bf16)
    wn1_lo = din("wn1_lo", [128, 128], bf16)
    wn1_hi = din("wn1_hi", [128, 128], bf16)
    bn1_col = din("bn1_col", [128, 1])
    wn2_d = din("wn2", [128, 128], bf16)
    bn2_row = din("bn2_row", [1, 128], bf16)
    ones_row = din("ones_row", [1, 128], bf16)
    iota128_d = din("iota128", [128, 128], bf16)
    iota_part_d = din("iota_part", [128, 1])
    ident_bf_d = din("ident_bf", [128, 128], bf16)
    colidx_d = din("colidx", [128, 64 * NP], i16)
    rowloc_c_d = din("rowloc_c", [128, 4 * NM])
    NPC3 = (NP + 2) // 3
    rowloc_p_d = din("rowloc_p", [128, NPC3 * 1024], bf16)
    allones_d = din("allones", [128, 128], bf16)
    out_d = nc.dram_tensor("out", [NWIN, 128, 128], f32, kind="ExternalOutput")

    B_hbm = nc.dram_tensor("B_scratch", [10240, 128], bf16)

    # flat macro list: (window, mw-in-window, nmw)
    macros = [(w, mw, mw_per_window[w])
              for w in range(NWIN) for mw in range(mw_per_window[w])]

    with tile.TileContext(nc) as tc:
        with (
            tc.tile_pool(name="persist", bufs=1) as pp,
            tc.tile_pool(name="work", bufs=bufs_w) as wp,
            tc.tile_pool(name="gout", bufs=bufs_g) as gp,
            tc.tile_pool(name="pse", bufs=3, space="PSUM") as pse,
            tc.tile_pool(name="psa", bufs=1, space="PSUM") as psa,
            tc.tile_pool(name="psn", bufs=1, space="PSUM") as psn,
        ):
            def load(t_dram, shape, dtype=f32):
                t = pp.tile(shape, dtype, tag=t_dram.name)
                nc.sync.dma_start(t[:], t_dram.ap())
                return t

            hT_t = load(hT_bf, [128, 10240], bf16)
            hTs_t = load(hTs_bf, [128, NWIN * 128], bf16)
            colidx_t = load(colidx_d, [128, 64 * NP], i16)
            rowloc_c = load(rowloc_c_d, [128, 4 * NM])
            w1t = load(we1_top, [128, 128], bf16)
            w1b = load(we1_bot, [128, 128], bf16)
            be1c = load(be1_col, [128, 1])
            w2 = load(we2_d, [128, 128], bf16)
            be2r = load(be2rep4, [1, 512], bf16)
            wn1l = load(wn1_lo, [128, 128], bf16)
            wn1h = load(wn1_hi, [128, 128], bf16)
            bn1c = load(bn1_col, [128, 1])
            wn2t = load(wn2_d, [128, 128], bf16)
            bn2r = load(bn2_row, [1, 128], bf16)
            onesr = load(ones_row, [1, 128], bf16)
            iota128 = load(iota128_d, [128, 128], bf16)
            iota_part = load(iota_part_d, [128, 1])
            ident_bf = load(ident_bf_d, [128, 128], bf16)
            rowloc_p = load(rowloc_p_d, [128, NPC3 * 1024], bf16)
            allones = load(allones_d, [128, 128], bf16)
            hsl_t = pp.tile([128, NWIN, 128], f32, tag="h_slice")
            nc.sync.dma_start(hsl_t[:], h_slice.ap().rearrange("w p d -> p w d"))

            # ---- B table: h @ we1_bot -> bf16 rows in HBM ----
            bview = B_hbm.ap().rearrange("(g t p) d -> g p t d", g=10, t=8, p=128)
            for g in range(10):
                stage = wp.tile([128, 8, 128], bf16, tag="bstage")
                for ts in range(8):
                    t = g * 8 + ts
                    bp = psn.tile([128, 512], f32, tag="tb")
                    nc.tensor.matmul(bp[:, 0:128], hT_t[:, t * 128:(t + 1) * 128],
                                     w1b[:], start=True, stop=True)
                    nc.vector.tensor_copy(stage[:, ts, :], bp[:, 0:128])
                nc.sync.dma_start(bview[g], stage[:])

            # ---- A table: h @ we1_top, bf16, SBUF resident (be1 in silu) ----
            a_t = pp.tile([128, NWIN, 128], bf16, tag="a_t")
            for w in range(NWIN):
                ap_ = psn.tile([128, 512], f32, tag="tb")
                nc.tensor.matmul(ap_[:, 0:128], hTs_t[:, w * 128:(w + 1) * 128],
                                 w1t[:], start=True, stop=True)
                nc.vector.tensor_copy(a_t[:, w, :], ap_[:, 0:128])

            agg_sb = pp.tile([128, NWIN, 128], bf16, tag="aggsb")

            gts = {}
            rbs = {}
            agg_tiles = {}
            stash = {}
            PREF = 6

            def issue_gather(cc):
                if cc >= NP:
                    return
                gt = gp.tile([128, 8, 128], bf16, tag="g")
                if not no_gather:
                    nc.gpsimd.dma_gather(
                        gt[:], B_hbm.ap(), colidx_t[:, cc * 64:(cc + 1) * 64],
                        num_idxs=CALL, num_idxs_reg=CALL, elem_size=128,
                        transpose=False, single_packet=True,
                        queue_num=QPAT[cc % len(QPAT)],
                    )
                else:
                    nc.vector.tensor_copy(gt[:, 0, 0:8], ident_bf[:, 0:8])
                gts[cc] = gt

            def node_phase(w):
                hp = psn.tile([128, 512], f32, tag="tb")
                nc.tensor.matmul(hp[:, 0:128], wn1l[:],
                                 hTs_t[:, w * 128:(w + 1) * 128],
                                 start=True, stop=False)
                nc.tensor.matmul(hp[:, 0:128], wn1h[:], agg_sb[:, w, :],
                                 start=False, stop=True)
                hs = wp.tile([128, 128], bf16, tag="hs")
                nc.scalar.activation(hs[:], hp[:, 0:128], AF.Silu,
                                     bias=bn1c[:, 0:1])
                op = psn.tile([128, 512], f32, tag="tb")
                nc.tensor.matmul(op[:, 0:128], onesr[:], bn2r[:],
                                 start=True, stop=False)
                nc.tensor.matmul(op[:, 0:128], hs[:], wn2t[:],
                                 start=False, stop=True)
                ot = wp.tile([128, 128], f32, tag="ot")
                nc.vector.tensor_tensor(ot[:], op[:, 0:128], hsl_t[:, w, :],
                                        ALU.add)
                nc.sync.dma_start(out_d.ap()[w], ot[:])

            def front(j):
                issue_gather(j + PREF)
                gt = gts.pop(j)
                if no_compute:
                    sink = wp.tile([128, 8], f32, tag="sink")
                    nc.vector.tensor_copy(sink[:], gt[:, 0, 0:8])
                    return
                st = wp.tile([128, 2, 512], bf16, tag="st")
                bp0 = 32 * (j % 3)
                cb0 = (j // 3) * 1024
                for h in range(2):
                    bc = psn.tile([128, 512], f32, tag="tb")
                    nc.tensor.matmul(
                        bc[:], allones[bp0:bp0 + 1, :],
                        rowloc_p[bp0:bp0 + 1, cb0 + h * 512:cb0 + (h + 1) * 512],
                        start=True, stop=True)
                    nc.vector.tensor_scalar(
                        st[:, h, :], bc[:], iota_part[:, 0:1],
                        None, ALU.is_equal)
                e1p = pse.tile([128, 1024], f32, tag="ep")
                for h in range(2):
                    w, mw, nmw = macros[2 * j + h]
                    sl = slice(h * 512, (h + 1) * 512)
                    nc.tensor.matmul(e1p[:, sl], a_t[:, w, :], st[:, h, :],
                                     start=True, stop=False,
                                     skip_group_check=True)
                    for t in range(4):
                        blk = h * 4 + t
                        nc.tensor.matmul(
                            e1p[:, blk * 128:(blk + 1) * 128],
                            gt[:, blk, :], ident_bf[:],
                            start=False, stop=True,
                            skip_group_check=True)
                e1s = wp.tile([128, 1024], bf16, tag="e1s")
                nc.scalar.activation(e1s[:], e1p[:], AF.Silu,
                                     bias=be1c[:, 0:1])
                stash[j] = e1s

            def back(j):
                e1s = stash.pop(j)
                s4 = wp.tile([128, 8, 128], bf16, tag="s4")
                for t in range(8):
                    m = 2 * j + t // 4
                    nc.vector.tensor_scalar(
                        s4[:, t, :], iota128[:],
                        rowloc_c[:, 4 * m + t % 4:4 * m + t % 4 + 1],
                        None, ALU.is_equal)
                e2p = pse.tile([128, 1024], f32, tag="ep")
                for h in range(2):
                    nc.tensor.matmul(e2p[:, h * 512:(h + 1) * 512],
                                     onesr[:], be2r[:],
                                     start=True, stop=False,
                                     skip_group_check=True)
                    for t in range(4):
                        blk = h * 4 + t
                        nc.tensor.matmul(
                            e2p[:, blk * 128:(blk + 1) * 128],
                            e1s[:, blk * 128:(blk + 1) * 128], w2[:],
                            start=False, stop=True, skip_group_check=True)
                e2s = wp.tile([128, 1024], bf16, tag="e2s")
                nc.scalar.activation(e2s[:], e2p[:], AF.Silu)
                for h in range(2):
                    w, mw, nmw = macros[2 * j + h]
                    if mw == 0:
                        agg_new = psa.tile([128, 128], f32, tag="agg")
                        agg_tiles[w] = agg_new
                    agg_ps = agg_tiles[w]
                    for t in range(4):
                        blk = h * 4 + t
                        nc.tensor.matmul(
                            agg_ps[:],
                            e2s[:, blk * 128:(blk + 1) * 128],
                            s4[:, blk, :],
                            start=(mw == 0 and t == 0),
                            stop=(mw == nmw - 1 and t == 3),
                            skip_group_check=True)
                    if mw == nmw - 1:
                        nc.vector.tensor_copy(agg_sb[:, w, :], agg_ps[:])
                        node_phase(w)

            if no_compute:
                for w in range(NWIN):
                    nc.sync.dma_start(out_d.ap()[w], hsl_t[:, w, :])
            for p in range(PREF):
                issue_gather(p)
            for j in range(NP + 1):
                if j < NP:
                    front(j)
                if j >= 1 and not no_compute:
                    back(j - 1)

    nc.compile()
    return nc


def _prep_inputs(h, edge_index, we1, be1, we2, be2, wn1, bn1, wn2, bn2):
    """Host-side shard/sort/pad. Returns (mw_per_window, per-core in_maps)."""
    h = np.asarray(h, np.float32)
    row = np.asarray(edge_index[0], np.int64).astype(np.int32)
    col = np.asarray(edge_index[1], np.int64).astype(np.int32)

    # per (core, window) edge lists
    core = row // NPC
    rl_g = row - core * NPC
    win = rl_g // 128
    rl = (rl_g % 128).astype(np.float32)

    counts = np.zeros((NCORES, NWIN), np.int64)
    per = [[None] * NWIN for _ in range(NCORES)]
    for cid in range(NCORES):
        msk = core == cid
        w_c, rl_c, col_c = win[msk], rl[msk], col[msk]
        for w in range(NWIN):
            wm = w_c == w
            per[cid][w] = (col_c[wm], rl_c[wm])
            counts[cid, w] = wm.sum()
    mw_w = [int(-(-counts[:, w].max() // MACRO)) for w in range(NWIN)]
    if sum(mw_w) % 2 == 1:
        mw_w[-1] += 1
    mw_per_window = tuple(mw_w)

    NM = sum(mw_per_window)
    NP = NM // 2

    hT_pad = np.zeros((128, 10240), np.float32)
    hT_pad[:, :N_NODES] = h.T
    bf = lambda x: np.asarray(x, np.float32).astype(BF16)
    shared = {
        "hT_bf": hT_pad.astype(BF16),
        "we1_top": bf(we1[:128]),
        "we1_bot": bf(we1[128:]),
        "be1_col": np.asarray(be1, np.float32)[:, None].copy(),
        "we2": bf(we2),
        "be2rep4": bf(np.tile(np.asarray(be2, np.float32), 4)[None, :]),
        "wn1_lo": bf(wn1[:128]),
        "wn1_hi": bf(np.asarray(wn1[128:], np.float32) / NORM),
        "bn1_col": np.asarray(bn1, np.float32)[:, None].copy(),
        "wn2": bf(wn2),
        "bn2_row": bf(np.asarray(bn2, np.float32)[None, :]),
        "ones_row": np.ones((1, 128), np.float32).astype(BF16),
        "iota128": bf(np.arange(128, dtype=np.float32)[None, :].repeat(128, 0)),
        "iota_part": np.arange(128, dtype=np.float32)[:, None].copy(),
        "ident_bf": np.eye(128, dtype=np.float32).astype(BF16),
        "allones": np.ones((128, 128), np.float32).astype(BF16),
    }

    in_maps = []
    for cid in range(NCORES):
        col_all = np.zeros(NM * 512, np.int32)
        rl_all = np.full(NM * 512, PAD_ROWLOCAL, np.float32)
        pos = 0
        for w in range(NWIN):
            ccol, crl = per[cid][w]
            col_all[pos:pos + len(ccol)] = ccol
            rl_all[pos:pos + len(crl)] = crl
            pos += mw_per_window[w] * 512
        colidx = np.zeros((128, 64 * NP), np.int16)
        for cc in range(NP):
            colidx[:, cc * 64:(cc + 1) * 64] = _wrap_idx16(
                col_all[cc * 1024:(cc + 1) * 1024])
        rowloc_c = np.zeros((128, 4 * NM), np.float32)
        rowloc_r = rl_all.reshape(NM, 512)
        for m in range(NM):
            rowloc_c[:, 4 * m:4 * m + 4] = rowloc_r[m].reshape(4, 128).T
        NPC3 = (NP + 2) // 3
        rowloc_p = np.full((128, NPC3 * 1024), PAD_ROWLOCAL, np.float32)
        rp = rl_all.reshape(NP, 1024)
        for j in range(NP):
            rowloc_p[32 * (j % 3),
                     (j // 3) * 1024:(j // 3 + 1) * 1024] = rp[j]
        base = cid * NPC
        hTs = hT_pad[:, base:base + NWIN * 128]
        h_slice = np.zeros((NWIN, 128, 128), np.float32)
        hi = min(N_NODES, base + NWIN * 128)
        h_slice.reshape(NWIN * 128, 128)[:hi - base] = h[base:hi]
        in_maps.append({**shared,
                        "hTs_bf": hTs.astype(BF16),
                        "h_slice": h_slice,
                        "colidx": colidx,
                        "rowloc_c": rowloc_c,
                        "rowloc_p": rowloc_p.astype(BF16)})
    return mw_per_window, in_maps


def kernel(**inputs):
    from concourse.bass_utils import run_bass_kernel_spmd

    mw, in_maps = _prep_inputs(**inputs)
    if mw not in _prog_cache:
        _prog_cache[mw] = _build_program(mw)
    nc = _prog_cache[mw]
    res = run_bass_kernel_spmd(nc, in_maps, list(range(NCORES)))
    outs = []
    for cid in range(NCORES):
        o = res.results[cid]["out"].reshape(NWIN * 128, 128)
        outs.append(o[:NPC])
    return np.concatenate(outs, axis=0)[:N_NODES].astype(np.float32)
